# revision 22
# baseline (speedup 1.0000x reference)
"""Trainium2 Bass kernel for the CLAPP 2-layer spiking net (T=100, B=256, I=700, H=512).

Data-parallel over batch across 8 cores (B=32/core). Per core:
  - cur = inp @ W.T precomputed as big PE matmuls (fp32 for layer 0; exact
    bf16 hi/lo split for layer 1 since spikes are {0,1}).
  - Sequential T-scan is 2 DVE ops/step on a [128, HC*B] state slice
    (layout: partition = h%128, free = (h//128)*B + b):
      u_t   = beta*u_{t-1} + w_t      (w_t = cur_t - beta*spk_{t-2}, GpSimd)
      spk_t = (u_t - 1) > spk_{t-1}   (reset_t == spk_{t-1})
    then one bulk GpSimd pass turns u into mem_t = u_t - spk_{t-1} in place.
  - Surrogate r = 1/(1 + pi^2 (mem-1)^2) via 3 whole-tensor ACT ops:
    Square(pi*mem - pi) -> Ln(s+1) -> Exp(-y). (ACT Reciprocal is banned.)
  - dW[b] = diag(prev/pi) @ ((L.T @ r[b]).T @ X[b]) with L[t,s]=beta^(t-s)[s<=t],
    X = layer input (inp / spk0): per-sample PE matmuls contracting over T,
    emitted in groups of 8 samples to keep the PE dense (HAM warm).
    prev/pi applied as per-partition ACT/DVE scale during PSUM eviction.
  - G/dW matmuls in float32r (1 cyc/row at N>=256).
  - tr1 and both losses recomputed on host from outputs (exact, cheap).
"""

import contextlib
import math

import numpy as np
import ml_dtypes

import concourse.bass as bass
from concourse import bacc
import concourse.mybir as mybir
import concourse.tile as tile
from concourse.bass_utils import run_bass_kernel_spmd

F32 = mybir.dt.float32
F32R = mybir.dt.float32r
BF16 = mybir.dt.bfloat16
AF = mybir.ActivationFunctionType
OP = mybir.AluOpType

BETA = 0.95
PI = math.pi
N_CORES = 8

DW_USE_F32R = True
DEBUG = False


def _ceil_div(a, b):
    return (a + b - 1) // b


def _splits(n, cap=512):
    ns = _ceil_div(n, cap)
    base, rem = divmod(n, ns)
    out, off = [], 0
    for j in range(ns):
        sz = base + (1 if j < rem else 0)
        out.append((off, sz))
        off += sz
    return out


def build_nc(T_, B, I, H0, H1):
    KI = _ceil_div(I, 128)
    KP = I - 128 * (KI - 1)
    HC0, HC1 = H0 // 128, H1 // 128
    SW0, SW1 = HC0 * B, HC1 * B
    assert SW0 <= 128 and SW1 <= 128
    NTB = T_ * B
    TBLK = max(1, 512 // B)
    ISPL0, ISPL1 = _splits(I), _splits(H0)
    MMF = F32R if DW_USE_F32R else F32

    def mm(ap):
        return ap

    nc = bacc.Bacc("TRN2", target_bir_lowering=False)

    d_inpT = nc.dram_tensor("inpT", [KI, 128, NTB], F32, kind="ExternalInput")
    d_inp = nc.dram_tensor("inp", [T_, B, I], MMF, kind="ExternalInput")
    d_w0t = nc.dram_tensor("w0t", [KI, 128, H0], F32, kind="ExternalInput")
    d_w1thi = nc.dram_tensor("w1thi", [HC0, 128, H1], BF16, kind="ExternalInput")
    d_w1tlo = nc.dram_tensor("w1tlo", [HC0, 128, H1], BF16, kind="ExternalInput")
    d_ppi0 = nc.dram_tensor("ppi0", [128, SW0], F32, kind="ExternalInput")
    d_ppi1 = nc.dram_tensor("ppi1", [128, SW1], F32, kind="ExternalInput")
    d_lmat = nc.dram_tensor("lmat", [T_, T_], MMF, kind="ExternalInput")
    d_idf = nc.dram_tensor("idf", [128, 128], F32, kind="ExternalInput")
    d_idb = nc.dram_tensor("idb", [128, 128], BF16, kind="ExternalInput")

    if DEBUG:
        d_dbg_cur0 = nc.dram_tensor("dbg_cur0", [128, T_ * SW0], F32, kind="ExternalOutput")
        d_dbg_mem0 = nc.dram_tensor("dbg_mem0", [128, T_ * SW0], F32, kind="ExternalOutput")
        d_dbg_r0 = nc.dram_tensor("dbg_r0", [128, T_ * SW0], F32, kind="ExternalOutput")
        d_dbg_spk0 = nc.dram_tensor("dbg_spk0", [128, T_ * SW0], BF16, kind="ExternalOutput")
    d_dW0 = nc.dram_tensor("dW0", [B, H0, I], F32, kind="ExternalOutput")
    d_dW1 = nc.dram_tensor("dW1", [B, H1, H0], F32, kind="ExternalOutput")
    d_spks1 = nc.dram_tensor("spks1", [T_, B, H1], F32, kind="ExternalOutput")
    d_tr0 = nc.dram_tensor("tr0", [B, H0], F32, kind="ExternalOutput")

    with tile.TileContext(nc) as tc:
        rr = [0]

        def evict(out_ap, in_ap, scale=None):
            rr[0] ^= 1
            if scale is None:
                if rr[0]:
                    nc.scalar.copy(out_ap, in_ap)
                else:
                    nc.vector.tensor_copy(out_ap, in_ap)
            elif rr[0]:
                nc.scalar.activation(out_ap, in_ap, AF.Copy, scale=scale)
            else:
                nc.vector.tensor_scalar(out_ap, in_ap, scale, None, op0=OP.mult)

        def scan(Tn, SW, cur_ap, u_ap, spk_ap, zmem, zspk, p_w):
            """2 DVE ops/step; w = cur - beta*spk_{t-2} precomputed on GpSimd.
            Afterwards a bulk GpSimd pass turns u into mem_t in place."""
            for t in range(Tn):
                sl = slice(t * SW, (t + 1) * SW)
                if t < 2:
                    w_ap = cur_ap[:, sl]
                else:
                    wt = p_w.tile([128, SW], F32, tag="w", name=f"w{t}")
                    nc.gpsimd.tensor_scalar(
                        wt[:], spk_ap[:, (t - 2) * SW:(t - 1) * SW], -BETA, None,
                        op0=OP.mult)
                    nc.gpsimd.tensor_tensor(
                        wt[:], wt[:], cur_ap[:, sl], op=OP.add)
                    w_ap = wt[:]
                nc.vector.scalar_tensor_tensor(
                    u_ap[:, sl], zmem if t == 0 else u_ap[:, (t - 1) * SW: t * SW],
                    BETA, w_ap, op0=OP.mult, op1=OP.add)
                nc.vector.scalar_tensor_tensor(
                    spk_ap[:, sl], u_ap[:, sl], 1.0,
                    zspk if t == 0 else spk_ap[:, (t - 1) * SW: t * SW],
                    op0=OP.subtract, op1=OP.is_gt)
            nc.gpsimd.tensor_tensor(
                u_ap[:, SW:Tn * SW], u_ap[:, SW:Tn * SW],
                spk_ap[:, 0:(Tn - 1) * SW], op=OP.subtract)

        def surrogate(u_ap, total, bias_npi, bias_one, bias_zero):
            nc.scalar.activation(u_ap[:, 0:total], u_ap[:, 0:total], AF.Square,
                                 bias=bias_npi, scale=PI)
            nc.scalar.activation(u_ap[:, 0:total], u_ap[:, 0:total], AF.Ln,
                                 bias=bias_one, scale=1.0)
            nc.scalar.activation(u_ap[:, 0:total], u_ap[:, 0:total], AF.Exp,
                                 bias=bias_zero, scale=-1.0)

        es = contextlib.ExitStack()
        with es:
            p_const = es.enter_context(tc.tile_pool(name="const", bufs=1))
            ps_tr = es.enter_context(tc.tile_pool(name="ps_tr", bufs=3, space="PSUM"))
            p_big = es.enter_context(tc.tile_pool(name="big", bufs=2))

            t_idf = p_const.tile([128, 128], F32)
            nc.sync.dma_start(t_idf[:], d_idf[:])
            t_idb = p_const.tile([128, 128], BF16)
            nc.sync.dma_start(t_idb[:], d_idb[:])
            t_lmat = p_const.tile([T_, T_], MMF)
            nc.sync.dma_start(t_lmat[:], d_lmat[:])
            t_ppi0 = p_const.tile([128, SW0], F32)
            nc.sync.dma_start(t_ppi0[:], d_ppi0[:])
            t_ppi1 = p_const.tile([128, SW1], F32)
            nc.sync.dma_start(t_ppi1[:], d_ppi1[:])
            SWM = max(SW0, SW1)
            t_zmem = p_const.tile([128, SWM], F32)
            nc.vector.memset(t_zmem[:], 0.0)
            t_zspk = p_const.tile([128, SWM], BF16)
            nc.vector.memset(t_zspk[:], 0.0)
            t_bnpi = p_const.tile([128, 1], F32)
            nc.vector.memset(t_bnpi[:], -PI)
            t_bone = p_const.tile([128, 1], F32)
            nc.vector.memset(t_bone[:], 1.0)
            t_bzero = p_const.tile([128, 1], F32)
            nc.vector.memset(t_bzero[:], 0.0)
            SWB = max(SW0, SW1)

            # ---------------- cur0 = inp @ W0.T (stream inpT) ----------------
            with tc.tile_pool(name="w0", bufs=1) as p_w0, \
                 tc.tile_pool(name="inprot", bufs=3) as p_inrot, \
                 tc.tile_pool(name="ps_mm0", bufs=4, space="PSUM") as ps_mm:
                t_w0t = p_w0.tile([128, KI * H0], F32)
                for k in range(KI):
                    nc.sync.dma_start(t_w0t[:, k * H0:(k + 1) * H0], d_w0t[k])

                t_cur0 = p_big.tile([128, T_ * SWB], F32, tag="big")
                cur0_v = t_cur0[:].rearrange("p (t hc b) -> p t hc b", hc=HC0, b=B)
                for t0 in range(0, T_, TBLK):
                    tcnt = min(TBLK, T_ - t0)
                    ncols = tcnt * B
                    psl = [ps_mm.tile([128, 512], F32, tag="mm", name=f"mmps{h}")
                           for h in range(HC0)]
                    for k in range(KI):
                        kp = KP if k == KI - 1 else 128
                        t_in = p_inrot.tile([128, TBLK * B], F32, tag="in")
                        nc.sync.dma_start(t_in[0:kp, 0:ncols],
                                          d_inpT[k, 0:kp, t0 * B: t0 * B + ncols])
                        for hc in range(HC0):
                            nc.tensor.matmul(
                                psl[hc][:, 0:ncols],
                                t_w0t[0:kp, k * H0 + hc * 128: k * H0 + (hc + 1) * 128],
                                t_in[0:kp, 0:ncols],
                                start=(k == 0), stop=(k == KI - 1))
                    for hc in range(HC0):
                        evict(cur0_v[:, t0:t0 + tcnt, hc, :],
                              psl[hc][:, 0:ncols].rearrange("p (t b) -> p t b", b=B))
            if DEBUG:
                nc.sync.dma_start(d_dbg_cur0[:], t_cur0[:])

            # ---------------- scan 0 ----------------
            with tc.tile_pool(name="spk0p", bufs=1) as p_spk0:
                t_u0 = p_big.tile([128, T_ * SWB], F32, tag="big")
                t_spk0 = p_spk0.tile([128, T_ * SW0], BF16)
                u0, spk0 = t_u0[:], t_spk0[:]
                with tc.tile_pool(name="w0rot", bufs=4) as p_w:
                    scan(T_, SW0, t_cur0[:], u0, spk0,
                         t_zmem[:, 0:SW0], t_zspk[:, 0:SW0], p_w)
                if DEBUG:
                    nc.sync.dma_start(d_dbg_mem0[:], u0)
                    nc.sync.dma_start(d_dbg_spk0[:], spk0)

                # ---------------- surrogate r0 ----------------
                surrogate(u0, T_ * SW0, t_bnpi[:], t_bone[:], t_bzero[:])
                if DEBUG:
                    nc.sync.dma_start(d_dbg_r0[:], u0)

                # ---------------- spk0T (bf16) + tr0 ----------------
                t_spk0T = p_big.tile([T_, B * H0], BF16, tag="spk0T")
                spk0_4d = spk0.rearrange("p (t hc b) -> p t hc b", hc=HC0, b=B)
                for b in range(B):
                    for hc in range(HC0):
                        pst = ps_tr.tile([T_, 128], BF16, tag="tr")
                        nc.tensor.transpose(pst[:], spk0_4d[:, :, hc, b], t_idb[:])
                        evict(t_spk0T[:, b * H0 + hc * 128: b * H0 + (hc + 1) * 128],
                              pst[:])

                t_trc = p_const.tile([128, SW0], F32, tag="trc")
                nc.vector.tensor_reduce(
                    t_trc[:], spk0.rearrange("p (t s) -> p s t", s=SW0),
                    axis=mybir.AxisListType.X, op=OP.add)
                ps_trc = ps_tr.tile([SW0, 128], F32, tag="tr")
                nc.tensor.transpose(ps_trc[:], t_trc[:], t_idf[:])
                t_trs = p_const.tile([SW0, 128], F32, tag="trs")
                nc.scalar.activation(t_trs[:], ps_trc[:], AF.Copy, scale=1.0 / T_)
                for hc in range(HC0):
                    nc.sync.dma_start(d_tr0[:, hc * 128:(hc + 1) * 128],
                                      t_trs[hc * B:(hc + 1) * B, :])

                # ---------------- cur1 = spk0 @ W1.T (bf16 hi/lo) ----------
                with tc.tile_pool(name="w1", bufs=1) as p_w1, \
                     tc.tile_pool(name="ps_mm1", bufs=2, space="PSUM") as ps_mm:
                    t_w1thi = p_w1.tile([128, HC0 * H1], BF16)
                    t_w1tlo = p_w1.tile([128, HC0 * H1], BF16)
                    for k in range(HC0):
                        nc.sync.dma_start(t_w1thi[:, k * H1:(k + 1) * H1], d_w1thi[k])
                        nc.sync.dma_start(t_w1tlo[:, k * H1:(k + 1) * H1], d_w1tlo[k])
                    t_cur1 = p_big.tile([128, T_ * SWB], F32, tag="big")
                    cur1_v = t_cur1[:].rearrange("p (t hc b) -> p t hc b", hc=HC1, b=B)
                    for hc in range(HC1):
                        for t0 in range(0, T_, TBLK):
                            tcnt = min(TBLK, T_ - t0)
                            ncols = tcnt * B
                            ps = ps_mm.tile([128, 512], F32, tag="mm")
                            nmm = 0
                            for k in range(HC0):
                                for w in (t_w1thi, t_w1tlo):
                                    nc.tensor.matmul(
                                        ps[:, 0:ncols],
                                        w[:, k * H1 + hc * 128: k * H1 + (hc + 1) * 128],
                                        spk0_4d[:, t0:t0 + tcnt, k, :],
                                        start=(nmm == 0), stop=(nmm == 2 * HC0 - 1))
                                    nmm += 1
                            evict(cur1_v[:, t0:t0 + tcnt, hc, :],
                                  ps[:, 0:ncols].rearrange("p (t b) -> p t b", b=B))

            # ---------------- dW0 pipeline (grouped for PE density) --------
            r0_4d = u0.rearrange("p (t hc b) -> p t hc b", hc=HC0, b=B)
            NBG = 4
            with tc.tile_pool(name="dw0rot", bufs=5) as p_rot, \
                 tc.tile_pool(name="dw0st", bufs=2) as p_st, \
                 tc.tile_pool(name="ps_g0", bufs=2, space="PSUM") as ps_g, \
                 tc.tile_pool(name="ps_dw0", bufs=3, space="PSUM") as ps_dw:
                for g0 in range(0, B, NBG):
                    grp = range(g0, min(B, g0 + NBG))
                    inpbs, rTs, ats = {}, {}, {}
                    for b in grp:
                        t_inpb = p_rot.tile([T_, I], MMF, tag="inpb", name=f"inpb{b}")
                        nc.sync.dma_start(t_inpb[:], d_inp[:, b, :])
                        inpbs[b] = t_inpb
                    for b in grp:
                        t_rT = p_rot.tile([T_, H0], MMF, tag="rT", name=f"rT{b}")
                        rTs[b] = t_rT
                        for hc in range(HC0):
                            pst = ps_tr.tile([T_, 128], F32, tag="tr")
                            nc.tensor.transpose(pst[:], r0_4d[:, :, hc, b], t_idf[:])
                            evict(t_rT[:, hc * 128:(hc + 1) * 128], pst[:])
                    for b in grp:
                        psg = ps_g.tile([T_, H0], F32, tag="g")
                        nc.tensor.matmul(psg[:], mm(t_lmat[:]), mm(rTs[b][:]),
                                         start=True, stop=True)
                        t_at = p_rot.tile([T_, H0], MMF, tag="at", name=f"at{b}")
                        ats[b] = t_at
                        evict(t_at[:], psg[:])
                    for b in grp:
                        for hc in range(HC0):
                            t_stg = p_st.tile([128, I], F32, tag="stg")
                            for (io, isz) in ISPL0:
                                psd = ps_dw.tile([128, 512], F32, tag="dw")
                                nc.tensor.matmul(
                                    psd[:, 0:isz],
                                    mm(ats[b][:, hc * 128:(hc + 1) * 128]),
                                    mm(inpbs[b][:, io:io + isz]),
                                    start=True, stop=True)
                                evict(t_stg[:, io:io + isz], psd[:, 0:isz],
                                      scale=t_ppi0[:, hc * B + b: hc * B + b + 1])
                            nc.gpsimd.dma_start(
                                d_dW0[b, hc * 128:(hc + 1) * 128, :], t_stg[:])

            # ---------------- scan 1 + spks1 out ----------------
            t_u1 = p_big.tile([128, T_ * SWB], F32, tag="big")
            u1 = t_u1[:]
            TOB = 8
            with tc.tile_pool(name="s1p", bufs=1) as p_s1f, \
                 tc.tile_pool(name="s1st", bufs=2) as p_s1st:
                t_spk1 = p_s1f.tile([128, T_ * SW1], BF16)
                spk1 = t_spk1[:]
                with tc.tile_pool(name="w1rot", bufs=4) as p_w:
                    scan(T_, SW1, t_cur1[:], u1, spk1,
                         t_zmem[:, 0:SW1], t_zspk[:, 0:SW1], p_w)
                for t0 in range(0, T_, TOB):
                    tcn = min(TOB, T_ - t0)
                    t_st = p_s1st.tile([SW1, TOB * 128], F32, tag="s1stage")
                    for j in range(tcn):
                        t = t0 + j
                        pso = ps_tr.tile([SW1, 128], BF16, tag="tr")
                        nc.tensor.transpose(
                            pso[:], spk1[:, t * SW1:(t + 1) * SW1], t_idb[:])
                        evict(t_st[:, j * 128:(j + 1) * 128], pso[:])
                    for hc in range(HC1):
                        nc.gpsimd.dma_start(
                            d_spks1[t0:t0 + tcn, :, hc * 128:(hc + 1) * 128]
                            .rearrange("t b f -> b t f"),
                            t_st[hc * B:(hc + 1) * B, 0:tcn * 128]
                            .rearrange("p (t f) -> p t f", f=128))

            # ---------------- surrogate r1 + dW1 pipeline ----------------
            surrogate(u1, T_ * SW1, t_bnpi[:], t_bone[:], t_bzero[:])
            r1_4d = u1.rearrange("p (t hc b) -> p t hc b", hc=HC1, b=B)
            with tc.tile_pool(name="dw1rot", bufs=5) as p_rot1, \
                 tc.tile_pool(name="dw1st", bufs=2) as p_st1, \
                 tc.tile_pool(name="ps_g1", bufs=2, space="PSUM") as ps_g, \
                 tc.tile_pool(name="ps_dw1", bufs=3, space="PSUM") as ps_dw:
                for g0 in range(0, B, NBG):
                    grp = range(g0, min(B, g0 + NBG))
                    rhsbs, rT1s, at1s = {}, {}, {}
                    for b in grp:
                        t_rhsb = p_rot1.tile([T_, H0], MMF, tag="rhsb", name=f"rhsb{b}")
                        nc.vector.tensor_copy(t_rhsb[:],
                                              t_spk0T[:, b * H0:(b + 1) * H0])
                        rhsbs[b] = t_rhsb
                        t_rT1 = p_rot1.tile([T_, H1], MMF, tag="rT1", name=f"rT1{b}")
                        rT1s[b] = t_rT1
                        for hc in range(HC1):
                            pst = ps_tr.tile([T_, 128], F32, tag="tr")
                            nc.tensor.transpose(pst[:], r1_4d[:, :, hc, b], t_idf[:])
                            evict(t_rT1[:, hc * 128:(hc + 1) * 128], pst[:])
                    for b in grp:
                        psg = ps_g.tile([T_, H1], F32, tag="g")
                        nc.tensor.matmul(psg[:], mm(t_lmat[:]), mm(rT1s[b][:]),
                                         start=True, stop=True)
                        t_at1 = p_rot1.tile([T_, H1], MMF, tag="at1", name=f"at1{b}")
                        at1s[b] = t_at1
                        evict(t_at1[:], psg[:])
                    for b in grp:
                        for hc in range(HC1):
                            t_stg = p_st1.tile([128, H0], F32, tag="stg1")
                            for (io, isz) in ISPL1:
                                psd = ps_dw.tile([128, 512], F32, tag="dw")
                                nc.tensor.matmul(
                                    psd[:, 0:isz],
                                    mm(at1s[b][:, hc * 128:(hc + 1) * 128]),
                                    mm(rhsbs[b][:, io:io + isz]),
                                    start=True, stop=True)
                                evict(t_stg[:, io:io + isz], psd[:, 0:isz],
                                      scale=t_ppi1[:, hc * B + b: hc * B + b + 1])
                            nc.gpsimd.dma_start(
                                d_dW1[b, hc * 128:(hc + 1) * 128, :], t_stg[:])
    nc.compile()
    return nc


def _host_prep(inp_s, W0, W1, prev0_s, prev1_s, T_, B, I, H0, H1, shared):
    KI = _ceil_div(I, 128)
    HC0, HC1 = H0 // 128, H1 // 128
    NTB = T_ * B
    x = np.ascontiguousarray(inp_s.transpose(2, 0, 1).reshape(I, NTB))
    xp = np.zeros((KI * 128, NTB), np.float32)
    xp[:I] = x
    inpT = xp.reshape(KI, 128, NTB)

    def ppi(prev, HC):
        p = (prev.T / PI).astype(np.float32)          # [H, B]
        return np.ascontiguousarray(
            p.reshape(HC, 128, B).transpose(1, 0, 2).reshape(128, HC * B))

    m = dict(shared)
    m.update(inpT=inpT, inp=np.ascontiguousarray(inp_s),
             ppi0=ppi(prev0_s, HC0), ppi1=ppi(prev1_s, HC1))
    return m


def _shared_prep(W0, W1, T_, I, H0, H1):
    KI = _ceil_div(I, 128)
    HC0 = H0 // 128
    H1 = W1.shape[0]
    w0t = np.zeros((KI * 128, H0), np.float32)
    w0t[:I] = W0.T
    w0t = np.ascontiguousarray(w0t.reshape(KI, 128, H0))
    w1t = np.ascontiguousarray(W1.T).astype(np.float32)   # [H0, H1]
    w1thi = w1t.astype(ml_dtypes.bfloat16)
    w1tlo = (w1t - w1thi.astype(np.float32)).astype(ml_dtypes.bfloat16)
    w1thi = np.ascontiguousarray(w1thi.reshape(HC0, 128, H1))
    w1tlo = np.ascontiguousarray(w1tlo.reshape(HC0, 128, H1))
    tt = np.arange(T_)
    Lmat = np.where(tt[:, None] >= tt[None, :],
                    np.float32(BETA) ** (tt[:, None] - tt[None, :]),
                    0.0).astype(np.float32)
    idf = np.eye(128, dtype=np.float32)
    idb = np.eye(128).astype(ml_dtypes.bfloat16)
    return dict(w0t=w0t, w1thi=w1thi, w1tlo=w1tlo, lmat=Lmat, idf=idf, idb=idb)


def run(inp, W0, W1, prev0, prev1, n_cores=N_CORES, trace=False):
    T_, Bfull, I = inp.shape
    H0, H1 = W0.shape[0], W1.shape[0]
    B = Bfull // n_cores
    nc = build_nc(T_, B, I, H0, H1)
    shared = _shared_prep(W0, W1, T_, I, H0, H1)
    in_maps = []
    for c in range(n_cores):
        sl = slice(c * B, (c + 1) * B)
        in_maps.append(_host_prep(inp[:, sl], W0, W1, prev0[sl], prev1[sl],
                                  T_, B, I, H0, H1, shared))
    res = run_bass_kernel_spmd(nc, in_maps, core_ids=list(range(n_cores)),
                               trace=trace)
    outs = res.results
    spks1 = np.concatenate([o["spks1"] for o in outs], axis=1)
    tr0 = np.concatenate([o["tr0"] for o in outs], axis=0)
    dW0 = np.concatenate([o["dW0"] for o in outs], axis=0)
    dW1 = np.concatenate([o["dW1"] for o in outs], axis=0)
    tr1 = spks1.sum(axis=0, dtype=np.float32) / np.float32(T_)
    fb0 = prev0 - prev0.mean(axis=-1, keepdims=True)
    fb1 = prev1 - prev1.mean(axis=-1, keepdims=True)
    loss0 = -(np.float32(T_) * (fb0 * tr0).sum(axis=-1)).mean(dtype=np.float32)
    loss1 = -(np.float32(T_) * (fb1 * tr1).sum(axis=-1)).mean(dtype=np.float32)
    losses = np.array([loss0, loss1], np.float32)
    spk_traces = np.stack([tr0, tr1]).astype(np.float32)
    return (spks1, spk_traces, losses, dW0, dW1), res


def kernel(inp, W0, W1, prev0, prev1, target=None, bf=None):
    inp = np.asarray(inp, np.float32)
    W0 = np.asarray(W0, np.float32)
    W1 = np.asarray(W1, np.float32)
    prev0 = np.asarray(prev0, np.float32)
    prev1 = np.asarray(prev1, np.float32)
    out, _ = run(inp, W0, W1, prev0, prev1)
    return out
